# revision 35
# baseline (speedup 1.0000x reference)
"""BinsChamferLoss Trainium2 Bass kernel.

Data-parallel over the batch: 8 samples -> 8 NeuronCores, one sample per core.
Each core computes its sample's chamfer terms (cham_x sum, masked cham_y sum,
valid count); the host combines the 8 per-sample scalars into the final loss.

Per-core algorithm (v1, brute force):
  points laid out [128 partitions x 600 free] (T = 76800)
  centers materialized [128 x 256] (edges host-replicated per partition)
  for each free column f: d2 = Square(centers - g[:, f]) via ACT per-partition
  bias; DVE reduce-min over centers -> cham_y column; GpSimd running min
  -> cham_x accumulator.  Invalid points are pushed to ~1e17 so they never
  win cham_x mins and their cham_y value is annihilated by the mask weight.
"""

import sys
from contextlib import ExitStack

import numpy as np

for _p in ("/opt/trn_rl_repo", "/root/.axon_site/_ro/trn_rl_repo"):
    if _p not in sys.path:
        sys.path.append(_p)

import concourse.tile as tile
from concourse import bacc, mybir
from concourse.bass_utils import run_bass_kernel_spmd

NCORES = 8
P, F = 128, 600          # per-core point layout, P*F = 76800
NB = 256                 # number of bins
NE = NB + 1              # bin edges
BIG = 1.0e17             # invalid-point displacement; BIG**2 stays finite in fp32

K = 2048                 # uniform grid cells over [0, 10)
SCALE = K / 10.0
NXB = 2048               # boundary grid built by matmul (4 x 512 PSUM chunks);
                         # tb[2048] = c_255 is patched with a copy
BL = 24                  # cham_x candidate block length (600 = 25*BL)
NBLK = F // BL
NCAND = P * NBLK         # 3200 candidates
VERSION = 2

_NC_CACHE = None


def _build():
    f32 = mybir.dt.float32
    op = mybir.AluOpType
    nc = bacc.Bacc(
        "TRN2", target_bir_lowering=False, debug=False, num_devices=NCORES
    )
    g_d = nc.dram_tensor("g", [P, F], f32, kind="ExternalInput").ap()
    m_d = nc.dram_tensor("mk", [P, F], f32, kind="ExternalInput").ap()
    e_d = nc.dram_tensor("edges", [P, NE], f32, kind="ExternalInput").ap()
    o_d = nc.dram_tensor("out", [1, 4], f32, kind="ExternalOutput").ap()

    with tile.TileContext(nc) as tc, ExitStack() as ctx:
        io = ctx.enter_context(tc.tile_pool(name="io", bufs=1))
        d2p = ctx.enter_context(tc.tile_pool(name="d2", bufs=4))

        # reload the gpsimd ucode first so it overlaps the whole table build
        nc.gpsimd.load_library(library_config.ap_gather)
        g = io.tile([P, F], f32)
        nc.sync.dma_start(g[:], g_d[:, :])
        mk = io.tile([P, F], f32)
        nc.sync.dma_start(mk[:], m_d[:, :])
        ed = io.tile([P, NE], f32)
        nc.sync.dma_start(ed[:], e_d[:, :])

        # centers = 0.5*(edges[1:] + edges[:-1]) on every partition
        cb = io.tile([P, NB], f32)
        nc.vector.tensor_tensor(cb[:], ed[:, 0:NB], ed[:, 1:NE], op=op.add)
        nc.vector.tensor_scalar_mul(cb[:], cb[:], 0.5)

        # ngx = -(mask ? g : ~BIG) = (-g) - (1-mk)*BIG, keeping the small and
        # huge scales apart so valid points stay exactly -g
        pen = io.tile([P, F], f32)
        nc.vector.tensor_scalar(
            pen[:], mk[:], -BIG, BIG, op0=op.mult, op1=op.add
        )
        ngx = io.tile([P, F], f32)
        nc.vector.scalar_tensor_tensor(
            ngx[:], g[:], -1.0, pen[:], op0=op.mult, op1=op.subtract
        )

        ymin = io.tile([P, F], f32)
        xacc = io.tile([P, NB], f32)
        nc.vector.memset(xacc[:], 3.0e38)

        for f in range(F):
            d2 = d2p.tile([P, NB], f32)
            nc.scalar.activation(
                d2[:], cb[:], mybir.ActivationFunctionType.Square,
                bias=ngx[:, f : f + 1], scale=1.0,
            )
            nc.vector.tensor_reduce(
                ymin[:, f : f + 1], d2[:], axis=mybir.AxisListType.X, op=op.min
            )
            nc.vector.tensor_tensor(xacc[:], xacc[:], d2[:], op=op.min)

        # masked cham_y sum and valid count, reduced along free dim
        wy = io.tile([P, F], f32)
        nc.vector.tensor_tensor(wy[:], ymin[:], mk[:], op=op.mult)
        ym2 = io.tile([P, 2], f32)
        nc.vector.tensor_reduce(
            ym2[:, 0:1], wy[:], axis=mybir.AxisListType.X, op=op.add
        )
        nc.vector.tensor_reduce(
            ym2[:, 1:2], mk[:], axis=mybir.AxisListType.X, op=op.add
        )

        # partition reductions on gpsimd (standard-library C-axis reduce)
        ym1 = io.tile([1, 2], f32)
        nc.gpsimd.tensor_reduce(
            ym1[:], ym2[:], axis=mybir.AxisListType.C, op=op.add
        )
        # cross-lane reduce supports only add/average/max: negate for the min
        nc.vector.tensor_scalar_mul(xacc[:], xacc[:], -1.0)
        xr = io.tile([1, NB], f32)
        nc.gpsimd.tensor_reduce(
            xr[:], xacc[:], axis=mybir.AxisListType.C, op=op.max
        )

        res = io.tile([1, 4], f32)
        nc.vector.memset(res[:], 0.0)
        nc.vector.tensor_reduce(
            res[0:1, 0:1], xr[:], axis=mybir.AxisListType.X, op=op.add,
            negate=True,
        )
        nc.vector.tensor_copy(res[0:1, 1:3], ym1[0:1, 0:2])
        nc.sync.dma_start(o_d[:, :], res[:])

    nc.compile()
    return nc


def _build_v2():
    """Grid-table kernel: nearest-center via uniform-cell two-candidate lookup.

    tb[j] = c[#midpoints <= j*delta] built as a PE matmul over the
    midpoint-vs-boundary step matrix; per-point candidates (tb[u], tb[u+1])
    fetched with one ap_gather each; cham_y = masked sum of min residual^2.
    cham_x: per-(partition, block) argmin candidates of the masked residuals,
    then exact 256 x NCAND brute force.
    """
    f32 = mybir.dt.float32
    i16 = mybir.dt.int16
    op = mybir.AluOpType
    AF = mybir.ActivationFunctionType
    from concourse import library_config

    nc = bacc.Bacc(
        "TRN2", target_bir_lowering=False, debug=False, num_devices=NCORES
    )
    g_d = nc.dram_tensor("g", [P, F], f32, kind="ExternalInput").ap()
    m_d = nc.dram_tensor("mk", [P, F], f32, kind="ExternalInput").ap()
    e_d = nc.dram_tensor("edges", [P, NE], f32, kind="ExternalInput").ap()
    xb_d = nc.dram_tensor("xb", [P, NXB], f32, kind="ExternalInput").ap()
    mn_d = nc.dram_tensor("mneg", [P, 16], f32, kind="ExternalInput").ap()
    ec_d = nc.dram_tensor("ecol", [P, 6], f32, kind="ExternalInput").ap()
    o_d = nc.dram_tensor("out", [1, 4], f32, kind="ExternalOutput").ap()
    cbs_d = nc.dram_tensor("cbs", [1, NCAND], f32).ap()

    with tile.TileContext(nc) as tc, ExitStack() as ctx:
        io = ctx.enter_context(tc.tile_pool(name="io", bufs=1))
        big = ctx.enter_context(tc.tile_pool(name="big", bufs=3))
        pp = ctx.enter_context(tc.tile_pool(name="pp", bufs=4, space="PSUM"))
        pps = ctx.enter_context(tc.tile_pool(name="pps", bufs=1, space="PSUM"))

        # reload the gpsimd ucode first so it overlaps the whole table build
        nc.gpsimd.load_library(library_config.ap_gather)
        g = io.tile([P, F], f32)
        nc.sync.dma_start(g[:], g_d[:, :])
        mk = io.tile([P, F], f32)
        nc.sync.dma_start(mk[:], m_d[:, :])
        ed = io.tile([P, NE], f32)
        nc.sync.dma_start(ed[:], e_d[:, :])
        mneg = io.tile([P, 16], f32)
        nc.sync.dma_start(mneg[:], mn_d[:, :])
        xb = big.tile([P, NXB], f32, tag="big")
        for q in range(4):
            q0, q1 = NXB * q // 4, NXB * (q + 1) // 4
            nc.sync.dma_start(xb[:, q0:q1], xb_d[:, q0:q1])

        # centers on every partition
        cb = io.tile([P, NB], f32)
        nc.vector.tensor_tensor(cb[:], ed[:, 0:NB], ed[:, 1:NE], op=op.add)
        nc.vector.tensor_scalar_mul(cb[:], cb[:], 0.5)

        # per-partition center columns from the host-transposed edge columns
        ec = io.tile([P, 6], f32)
        nc.sync.dma_start(ec[:], ec_d[:, :])
        ccA = io.tile([P, 1], f32)   # c_0..127
        nc.vector.tensor_tensor(ccA[:], ec[:, 0:1], ec[:, 1:2], op=op.add)
        nc.vector.tensor_scalar_mul(ccA[:], ccA[:], 0.5)
        ccB = io.tile([P, 1], f32)   # c_1..128
        nc.vector.tensor_tensor(ccB[:], ec[:, 1:2], ec[:, 2:3], op=op.add)
        nc.vector.tensor_scalar_mul(ccB[:], ccB[:], 0.5)
        ccC = io.tile([P, 1], f32)   # c_128..255
        nc.vector.tensor_tensor(ccC[:], ec[:, 3:4], ec[:, 4:5], op=op.add)
        nc.vector.tensor_scalar_mul(ccC[:], ccC[:], 0.5)
        ccD = io.tile([P, 1], f32)   # c_129..255, last lane pinned to c_255
        nc.vector.tensor_tensor(ccD[:], ec[:, 4:5], ec[:, 5:6], op=op.add)
        nc.vector.tensor_scalar_mul(ccD[:], ccD[:], 0.5)

        # midpoints and center deltas per partition (two 128-blocks)
        mv1 = io.tile([P, 1], f32)
        nc.vector.tensor_tensor(mv1[:], ccA[:], ccB[:], op=op.add)
        nc.vector.tensor_scalar_mul(mv1[:], mv1[:], 0.5)
        mv2 = io.tile([P, 1], f32)
        nc.vector.tensor_tensor(mv2[:], ccC[:], ccD[:], op=op.add)
        nc.vector.tensor_scalar_mul(mv2[:], mv2[:], 0.5)
        dcv1 = io.tile([P, 1], f32)
        nc.vector.tensor_tensor(dcv1[:], ccB[:], ccA[:], op=op.subtract)
        # dcv2[127] = c_255 - c_255 = 0, so the padded midpoint row is inert
        dcv2 = io.tile([P, 1], f32)
        nc.vector.tensor_tensor(dcv2[:], ccD[:], ccC[:], op=op.subtract)

        # fp16 matmul with Dekker hi/lo split of dc so the 255-term prefix
        # sums stay fp32-accurate while the matmul runs at fp16 rate
        f16 = mybir.dt.float16
        dch1 = io.tile([P, 1], f16)
        nc.vector.tensor_copy(dch1[:], dcv1[:])
        dch2 = io.tile([P, 1], f16)
        nc.vector.tensor_copy(dch2[:], dcv2[:])
        dlo1 = io.tile([P, 1], f32)
        nc.vector.tensor_tensor(dlo1[:], dcv1[:], dch1[:], op=op.subtract)
        dlo2 = io.tile([P, 1], f32)
        nc.vector.tensor_tensor(dlo2[:], dcv2[:], dch2[:], op=op.subtract)
        dcO1 = io.tile([P, P], f16)
        nc.vector.tensor_copy(dcO1[:], dch1[:].broadcast_to([P, P]))
        dcO2 = io.tile([P, P], f16)
        nc.vector.tensor_copy(dcO2[:], dch2[:].broadcast_to([P, P]))
        dcL1 = io.tile([P, P], f16)
        nc.vector.tensor_copy(dcL1[:], dlo1[:].broadcast_to([P, P]))
        dcL2 = io.tile([P, P], f16)
        nc.vector.tensor_copy(dcL2[:], dlo2[:].broadcast_to([P, P]))

        # step matrices over boundary grid
        M1 = big.tile([P, NXB], f16, tag="big")
        M2 = big.tile([P, NXB], f16, tag="big")
        for q in range(4):
            q0, q1 = NXB * q // 4, NXB * (q + 1) // 4
            nc.vector.tensor_scalar(
                M1[:, q0:q1], xb[:, q0:q1], mv1[:], None, op0=op.is_ge
            )
            nc.vector.tensor_scalar(
                M2[:, q0:q1], xb[:, q0:q1], mv2[:], None, op0=op.is_ge
            )

        # tb[j] = c0 + sum_q dc_q * M[q, j], broadcast on all partitions
        tbb = io.tile([P, NXB + 4], f32)
        # boundary j = K sits at exactly 10.0, above every midpoint
        nc.vector.tensor_copy(tbb[:, K : K + 1], cb[:, NB - 1 : NB])
        c0b = cb[:, 0:1]
        for k in range(NXB // 512):
            ps = pp.tile([P, 512], f32)
            nc.tensor.matmul(
                ps[:], dcO1[:], M1[:, 512 * k : 512 * (k + 1)],
                start=True, stop=False,
            )
            nc.tensor.matmul(
                ps[:], dcL1[:], M1[:, 512 * k : 512 * (k + 1)],
                start=False, stop=False,
            )
            nc.tensor.matmul(
                ps[:], dcO2[:], M2[:, 512 * k : 512 * (k + 1)],
                start=False, stop=False,
            )
            nc.tensor.matmul(
                ps[:], dcL2[:], M2[:, 512 * k : 512 * (k + 1)],
                start=False, stop=True,
            )
            nc.scalar.activation(
                tbb[:, 512 * k : 512 * (k + 1)], ps[:], AF.Identity,
                bias=c0b, scale=1.0,
            )

        # per-point cell index
        uf = io.tile([P, F], f32)
        nc.vector.tensor_scalar(
            uf[:], g[:], float(SCALE), -0.5, op0=op.mult, op1=op.add
        )
        u16 = io.tile([P, F], i16)
        nc.vector.tensor_scalar(
            u16[:], uf[:], float(K - 1), 0.0, op0=op.min, op1=op.max
        )

        # prep work that only needs g/mk: scheduled into the gather window
        gxp = io.tile([P, F], f32)
        nc.vector.tensor_scalar(
            gxp[:], mk[:], -BIG, BIG, op0=op.mult, op1=op.add
        )
        gx = io.tile([P, F], f32)
        nc.vector.tensor_tensor(gx[:], g[:], gxp[:], op=op.add)
        d2pen = io.tile([P, F], f32)
        nc.vector.tensor_scalar(
            d2pen[:], mk[:], -1.0e30, 1.0e30, op0=op.mult, op1=op.add
        )
        mlen = io.tile([P, 1], f32)
        nc.vector.tensor_reduce(
            mlen[:], mk[:], axis=mybir.AxisListType.X, op=op.add
        )

        # gather candidate centers tb[u], tb[u+1] in two BL-aligned f-halves;
        # each half's cham_x tail overlaps the other half's merges
        nccA = io.tile([P, 1], f32)
        nc.vector.tensor_scalar(nccA[:], ccA[:], -1.0, None, op0=op.mult)
        nccC = io.tile([P, 1], f32)
        nc.vector.tensor_scalar(nccC[:], ccC[:], -1.0, None, op0=op.mult)
        onesc = io.tile([P, 1], f32)
        nc.vector.memset(onesc[:], 1.0)

        HALVES = ((0, 216), (216, 432), (432, 600))
        NH = len(HALVES)
        ysums = io.tile([P, NH], f32)
        xmin4 = io.tile([P, 2 * NH], f32)   # column NH*b + h

        gts = []
        for f0, f1 in HALVES:
            fw = f1 - f0
            for tab0 in range(2):
                gt = big.tile([P, fw * 16], f32, tag="big")
                nc.gpsimd.ap_gather(
                    gt[:], tbb[:, tab0 : tab0 + K], u16[:, f0:f1],
                    channels=P, num_elems=K, d=1, num_idxs=fw * 16,
                )
                gts.append(gt)

        for h, (f0, f1) in enumerate(HALVES):
            fw = f1 - f0
            nb = fw // BL
            rLo = io.tile([P, fw], f32, tag=f"rlo{h}")
            nc.vector.tensor_copy(rLo[:], g[:, f0:f1])
            rHi = io.tile([P, fw], f32, tag=f"rhi{h}")
            nc.vector.tensor_copy(rHi[:], g[:, f0:f1])
            for tab0, dst in ((0, rLo), (1, rHi)):
                gv = gts[2 * h + tab0][:].rearrange("p (f r) -> p f r", r=16)
                for r in range(16):
                    nc.vector.scalar_tensor_tensor(
                        dst[:], gv[:, :, r], mneg[:, r : r + 1], dst[:],
                        op0=op.mult, op1=op.add,
                    )

            rLo2 = io.tile([P, fw], f32, tag=f"rl2{h}")
            nc.vector.tensor_tensor(rLo2[:], rLo[:], rLo[:], op=op.mult)
            rHi2 = io.tile([P, fw], f32, tag=f"rh2{h}")
            nc.vector.tensor_tensor(rHi2[:], rHi[:], rHi[:], op=op.mult)
            d2y = io.tile([P, fw], f32, tag=f"d2y{h}")
            nc.vector.tensor_tensor(d2y[:], rLo2[:], rHi2[:], op=op.min)

            junk = io.tile([P, fw], f32, tag="junk")
            nc.vector.scalar_tensor_tensor(
                junk[:], d2y[:], 1.0, mk[:, f0:f1], op0=op.mult, op1=op.mult,
                accum_out=ysums[:, h : h + 1],
            )

            d2m = io.tile([P, fw], f32, tag=f"d2m{h}")
            nc.vector.tensor_tensor(
                d2m[:], d2pen[:, f0:f1], d2y[:], op=op.add
            )
            d2mv = d2m[:].rearrange("p (b l) -> p b l", l=BL)
            gxv = gx[:, f0:f1].rearrange("p (b l) -> p b l", l=BL)
            m1t = io.tile([P, nb], f32, tag=f"m1t{h}")
            nc.vector.tensor_reduce(
                m1t[:], d2mv, axis=mybir.AxisListType.X, op=op.min
            )
            eqt = io.tile([P, fw], f32, tag=f"eqt{h}")
            eqv = eqt[:].rearrange("p (b l) -> p b l", l=BL)
            nc.vector.tensor_tensor(
                eqv, d2mv, m1t[:].unsqueeze(2).broadcast_to([P, nb, BL]),
                op=op.is_equal,
            )
            get = io.tile([P, fw], f32, tag=f"get{h}")
            gev = get[:].rearrange("p (b l) -> p b l", l=BL)
            nc.vector.tensor_tensor(gev, gxv, eqv, op=op.mult)
            gcand = io.tile([P, nb], f32, tag=f"gc{h}")
            nc.vector.tensor_reduce(
                gcand[:], gev, axis=mybir.AxisListType.X, op=op.max
            )

            off = (f0 // BL) * P
            ncand_h = nb * P
            nc.sync.dma_start(cbs_d[:, off : off + ncand_h], gcand[:])
            cbnd = io.tile([P, ncand_h], f32, tag=f"cbn{h}")
            nc.sync.dma_start(
                cbnd[:],
                cbs_d[:, off : off + ncand_h].broadcast_to([P, ncand_h]),
            )
            for b, ncc in ((0, nccA), (1, nccC)):
                d2c = big.tile([P, ncand_h], f32, tag="big")
                nc.scalar.activation(
                    d2c[:], cbnd[:], AF.Square, bias=ncc[:], scale=1.0
                )
                j = NH * b + h
                nc.vector.tensor_reduce(
                    xmin4[:, j : j + 1], d2c[:],
                    axis=mybir.AxisListType.X, op=op.min,
                )

        ysum = io.tile([P, 1], f32)
        nc.vector.tensor_reduce(
            ysum[:], ysums[:], axis=mybir.AxisListType.X, op=op.add
        )
        xmin = io.tile([P, 2], f32)
        nc.vector.tensor_reduce(
            xmin[:], xmin4[:].rearrange("p (b h) -> p b h", h=NH),
            axis=mybir.AxisListType.X, op=op.min,
        )

        # partition reductions via ones matmuls
        ps_y = pps.tile([1, 1], f32)
        nc.tensor.matmul(ps_y[:], ysum[:], onesc[:], start=True, stop=True)
        ps_m = pps.tile([1, 1], f32)
        nc.tensor.matmul(ps_m[:], mlen[:], onesc[:], start=True, stop=True)
        ps_x = pps.tile([1, 2], f32)
        nc.tensor.matmul(ps_x[:], onesc[:], xmin[:], start=True, stop=True)

        res = io.tile([1, 4], f32)
        nc.vector.memset(res[:], 0.0)
        xrow = io.tile([1, 2], f32)
        nc.vector.tensor_copy(xrow[:], ps_x[:])
        nc.vector.tensor_tensor(
            res[0:1, 0:1], xrow[0:1, 0:1], xrow[0:1, 1:2], op=op.add
        )
        nc.vector.tensor_copy(res[0:1, 1:2], ps_y[:])
        nc.vector.tensor_copy(res[0:1, 2:3], ps_m[:])
        nc.sync.dma_start(o_d[:, :], res[:])

    nc.compile()
    return nc


def _host_consts():
    xb = np.broadcast_to(
        (np.arange(NXB, dtype=np.float32) / np.float32(SCALE)).reshape(1, NXB),
        (P, NXB),
    )
    mneg = np.zeros((P, 16), dtype=np.float32)
    for p in range(P):
        mneg[p, p % 16] = -1.0
    return np.ascontiguousarray(xb), mneg


def _get_nc():
    global _NC_CACHE
    if _NC_CACHE is None:
        _NC_CACHE = _build_v2() if VERSION == 2 else _build()
    return _NC_CACHE


def kernel(depth_pred=None, depth_gt=None, depth_mask=None, bin_edges=None):
    nc = _get_nc()
    if VERSION == 2:
        xb, mneg = _host_consts()
    in_maps = []
    for n in range(NCORES):
        edges_rep = np.broadcast_to(
            bin_edges[n].reshape(1, NE).astype(np.float32), (P, NE)
        )
        im = {
            "g": np.ascontiguousarray(
                depth_gt[n].reshape(P, F).astype(np.float32)
            ),
            "mk": np.ascontiguousarray(
                depth_mask[n].reshape(P, F).astype(np.float32)
            ),
            "edges": np.ascontiguousarray(edges_rep),
        }
        if VERSION == 2:
            im["xb"] = xb
            im["mneg"] = mneg
            e = bin_edges[n].reshape(-1).astype(np.float32)
            ecol = np.empty((P, 6), dtype=np.float32)
            idx = np.arange(P)
            ecol[:, 0] = e[idx]
            ecol[:, 1] = e[idx + 1]
            ecol[:, 2] = e[idx + 2]
            ecol[:, 3] = e[np.minimum(idx + 128, NE - 2)]
            ecol[:, 4] = e[np.minimum(idx + 129, NE - 1)]
            ecol[:, 5] = e[np.minimum(idx + 130, NE - 1)]
            # pin the pad lane so ccD[127] = c_255 exactly
            ecol[127, 5] = e[255]
            im["ecol"] = ecol
        in_maps.append(im)
    res = run_bass_kernel_spmd(nc, in_maps, core_ids=list(range(NCORES)))
    per = np.empty(NCORES, dtype=np.float32)
    for n in range(NCORES):
        o = res.results[n]["out"].reshape(-1)
        per[n] = np.float32(o[0] / np.float32(NB)) + np.float32(o[1] / o[2])
    return np.float32(per.mean(dtype=np.float32))


# revision 36
# speedup vs baseline: 1.0397x; 1.0397x over previous
"""BinsChamferLoss Trainium2 Bass kernel.

Data-parallel over the batch: 8 samples -> 8 NeuronCores, one sample per core.
Each core computes its sample's chamfer terms (cham_x sum, masked cham_y sum,
valid count); the host combines the 8 per-sample scalars into the final loss.

Per-core algorithm (v1, brute force):
  points laid out [128 partitions x 600 free] (T = 76800)
  centers materialized [128 x 256] (edges host-replicated per partition)
  for each free column f: d2 = Square(centers - g[:, f]) via ACT per-partition
  bias; DVE reduce-min over centers -> cham_y column; GpSimd running min
  -> cham_x accumulator.  Invalid points are pushed to ~1e17 so they never
  win cham_x mins and their cham_y value is annihilated by the mask weight.
"""

import sys
from contextlib import ExitStack

import numpy as np

for _p in ("/opt/trn_rl_repo", "/root/.axon_site/_ro/trn_rl_repo"):
    if _p not in sys.path:
        sys.path.append(_p)

import concourse.tile as tile
from concourse import bacc, mybir
from concourse.bass_utils import run_bass_kernel_spmd

NCORES = 8
P, F = 128, 600          # per-core point layout, P*F = 76800
NB = 256                 # number of bins
NE = NB + 1              # bin edges
BIG = 1.0e17             # invalid-point displacement; BIG**2 stays finite in fp32

K = 2048                 # uniform grid cells over [0, 10)
SCALE = K / 10.0
NXB = 2048               # boundary grid built by matmul (4 x 512 PSUM chunks);
                         # tb[2048] = c_255 is patched with a copy
BL = 24                  # cham_x candidate block length (600 = 25*BL)
NBLK = F // BL
NCAND = P * NBLK         # 3200 candidates
VERSION = 2

_NC_CACHE = None


def _build():
    f32 = mybir.dt.float32
    op = mybir.AluOpType
    nc = bacc.Bacc(
        "TRN2", target_bir_lowering=False, debug=False, num_devices=NCORES
    )
    g_d = nc.dram_tensor("g", [P, F], f32, kind="ExternalInput").ap()
    m_d = nc.dram_tensor("mk", [P, F], f32, kind="ExternalInput").ap()
    e_d = nc.dram_tensor("edges", [P, NE], f32, kind="ExternalInput").ap()
    o_d = nc.dram_tensor("out", [1, 4], f32, kind="ExternalOutput").ap()

    with tile.TileContext(nc) as tc, ExitStack() as ctx:
        io = ctx.enter_context(tc.tile_pool(name="io", bufs=1))
        d2p = ctx.enter_context(tc.tile_pool(name="d2", bufs=4))

        # reload the gpsimd ucode first so it overlaps the whole table build
        nc.gpsimd.load_library(library_config.ap_gather)
        g = io.tile([P, F], f32)
        nc.sync.dma_start(g[:], g_d[:, :])
        mk = io.tile([P, F], f32)
        nc.sync.dma_start(mk[:], m_d[:, :])
        ed = io.tile([P, NE], f32)
        nc.sync.dma_start(ed[:], e_d[:, :])

        # centers = 0.5*(edges[1:] + edges[:-1]) on every partition
        cb = io.tile([P, NB], f32)
        nc.vector.tensor_tensor(cb[:], ed[:, 0:NB], ed[:, 1:NE], op=op.add)
        nc.vector.tensor_scalar_mul(cb[:], cb[:], 0.5)

        # ngx = -(mask ? g : ~BIG) = (-g) - (1-mk)*BIG, keeping the small and
        # huge scales apart so valid points stay exactly -g
        pen = io.tile([P, F], f32)
        nc.vector.tensor_scalar(
            pen[:], mk[:], -BIG, BIG, op0=op.mult, op1=op.add
        )
        ngx = io.tile([P, F], f32)
        nc.vector.scalar_tensor_tensor(
            ngx[:], g[:], -1.0, pen[:], op0=op.mult, op1=op.subtract
        )

        ymin = io.tile([P, F], f32)
        xacc = io.tile([P, NB], f32)
        nc.vector.memset(xacc[:], 3.0e38)

        for f in range(F):
            d2 = d2p.tile([P, NB], f32)
            nc.scalar.activation(
                d2[:], cb[:], mybir.ActivationFunctionType.Square,
                bias=ngx[:, f : f + 1], scale=1.0,
            )
            nc.vector.tensor_reduce(
                ymin[:, f : f + 1], d2[:], axis=mybir.AxisListType.X, op=op.min
            )
            nc.vector.tensor_tensor(xacc[:], xacc[:], d2[:], op=op.min)

        # masked cham_y sum and valid count, reduced along free dim
        wy = io.tile([P, F], f32)
        nc.vector.tensor_tensor(wy[:], ymin[:], mk[:], op=op.mult)
        ym2 = io.tile([P, 2], f32)
        nc.vector.tensor_reduce(
            ym2[:, 0:1], wy[:], axis=mybir.AxisListType.X, op=op.add
        )
        nc.vector.tensor_reduce(
            ym2[:, 1:2], mk[:], axis=mybir.AxisListType.X, op=op.add
        )

        # partition reductions on gpsimd (standard-library C-axis reduce)
        ym1 = io.tile([1, 2], f32)
        nc.gpsimd.tensor_reduce(
            ym1[:], ym2[:], axis=mybir.AxisListType.C, op=op.add
        )
        # cross-lane reduce supports only add/average/max: negate for the min
        nc.vector.tensor_scalar_mul(xacc[:], xacc[:], -1.0)
        xr = io.tile([1, NB], f32)
        nc.gpsimd.tensor_reduce(
            xr[:], xacc[:], axis=mybir.AxisListType.C, op=op.max
        )

        res = io.tile([1, 4], f32)
        nc.vector.memset(res[:], 0.0)
        nc.vector.tensor_reduce(
            res[0:1, 0:1], xr[:], axis=mybir.AxisListType.X, op=op.add,
            negate=True,
        )
        nc.vector.tensor_copy(res[0:1, 1:3], ym1[0:1, 0:2])
        nc.sync.dma_start(o_d[:, :], res[:])

    nc.compile()
    return nc


def _build_v2():
    """Grid-table kernel: nearest-center via uniform-cell two-candidate lookup.

    tb[j] = c[#midpoints <= j*delta] built as a PE matmul over the
    midpoint-vs-boundary step matrix; per-point candidates (tb[u], tb[u+1])
    fetched with one ap_gather each; cham_y = masked sum of min residual^2.
    cham_x: per-(partition, block) argmin candidates of the masked residuals,
    then exact 256 x NCAND brute force.
    """
    f32 = mybir.dt.float32
    i16 = mybir.dt.int16
    op = mybir.AluOpType
    AF = mybir.ActivationFunctionType
    from concourse import library_config

    nc = bacc.Bacc(
        "TRN2", target_bir_lowering=False, debug=False, num_devices=NCORES
    )
    g_d = nc.dram_tensor("g", [P, F], f32, kind="ExternalInput").ap()
    m_d = nc.dram_tensor("mk", [P, F], f32, kind="ExternalInput").ap()
    e_d = nc.dram_tensor("edges", [P, NE], f32, kind="ExternalInput").ap()
    xb_d = nc.dram_tensor("xb", [P, NXB], f32, kind="ExternalInput").ap()
    mn_d = nc.dram_tensor("mneg", [P, 16], f32, kind="ExternalInput").ap()
    ec_d = nc.dram_tensor("ecol", [P, 6], f32, kind="ExternalInput").ap()
    o_d = nc.dram_tensor("out", [1, 4], f32, kind="ExternalOutput").ap()
    cbs_d = nc.dram_tensor("cbs", [1, NCAND], f32).ap()

    with tile.TileContext(nc) as tc, ExitStack() as ctx:
        io = ctx.enter_context(tc.tile_pool(name="io", bufs=1))
        big = ctx.enter_context(tc.tile_pool(name="big", bufs=3))
        pp = ctx.enter_context(tc.tile_pool(name="pp", bufs=4, space="PSUM"))
        pps = ctx.enter_context(tc.tile_pool(name="pps", bufs=1, space="PSUM"))

        # reload the gpsimd ucode first so it overlaps the whole table build
        nc.gpsimd.load_library(library_config.ap_gather)
        # table-build inputs first: the SP sequencer issues DMAs serially
        # (~565ns each) and ecol/xb gate the critical chain
        ec = io.tile([P, 6], f32)
        nc.sync.dma_start(ec[:], ec_d[:, :])
        xb = big.tile([P, NXB], f32, tag="big")
        for q in range(4):
            q0, q1 = NXB * q // 4, NXB * (q + 1) // 4
            nc.sync.dma_start(xb[:, q0:q1], xb_d[:, q0:q1])
        ed = io.tile([P, NE], f32)
        nc.sync.dma_start(ed[:], e_d[:, :])
        g = io.tile([P, F], f32)
        nc.sync.dma_start(g[:], g_d[:, :])
        mk = io.tile([P, F], f32)
        nc.sync.dma_start(mk[:], m_d[:, :])
        mneg = io.tile([P, 16], f32)
        nc.sync.dma_start(mneg[:], mn_d[:, :])

        # centers on every partition
        cb = io.tile([P, NB], f32)
        nc.vector.tensor_tensor(cb[:], ed[:, 0:NB], ed[:, 1:NE], op=op.add)
        nc.vector.tensor_scalar_mul(cb[:], cb[:], 0.5)

        # per-partition center columns from the host-transposed edge columns
        ccA = io.tile([P, 1], f32)   # c_0..127
        nc.vector.tensor_tensor(ccA[:], ec[:, 0:1], ec[:, 1:2], op=op.add)
        nc.vector.tensor_scalar_mul(ccA[:], ccA[:], 0.5)
        ccB = io.tile([P, 1], f32)   # c_1..128
        nc.vector.tensor_tensor(ccB[:], ec[:, 1:2], ec[:, 2:3], op=op.add)
        nc.vector.tensor_scalar_mul(ccB[:], ccB[:], 0.5)
        ccC = io.tile([P, 1], f32)   # c_128..255
        nc.vector.tensor_tensor(ccC[:], ec[:, 3:4], ec[:, 4:5], op=op.add)
        nc.vector.tensor_scalar_mul(ccC[:], ccC[:], 0.5)
        ccD = io.tile([P, 1], f32)   # c_129..255, last lane pinned to c_255
        nc.vector.tensor_tensor(ccD[:], ec[:, 4:5], ec[:, 5:6], op=op.add)
        nc.vector.tensor_scalar_mul(ccD[:], ccD[:], 0.5)

        # midpoints and center deltas per partition (two 128-blocks)
        mv1 = io.tile([P, 1], f32)
        nc.vector.tensor_tensor(mv1[:], ccA[:], ccB[:], op=op.add)
        nc.vector.tensor_scalar_mul(mv1[:], mv1[:], 0.5)
        mv2 = io.tile([P, 1], f32)
        nc.vector.tensor_tensor(mv2[:], ccC[:], ccD[:], op=op.add)
        nc.vector.tensor_scalar_mul(mv2[:], mv2[:], 0.5)
        dcv1 = io.tile([P, 1], f32)
        nc.vector.tensor_tensor(dcv1[:], ccB[:], ccA[:], op=op.subtract)
        # dcv2[127] = c_255 - c_255 = 0, so the padded midpoint row is inert
        dcv2 = io.tile([P, 1], f32)
        nc.vector.tensor_tensor(dcv2[:], ccD[:], ccC[:], op=op.subtract)

        # fp16 matmul with Dekker hi/lo split of dc so the 255-term prefix
        # sums stay fp32-accurate while the matmul runs at fp16 rate
        f16 = mybir.dt.float16
        dch1 = io.tile([P, 1], f16)
        nc.vector.tensor_copy(dch1[:], dcv1[:])
        dch2 = io.tile([P, 1], f16)
        nc.vector.tensor_copy(dch2[:], dcv2[:])
        dlo1 = io.tile([P, 1], f32)
        nc.vector.tensor_tensor(dlo1[:], dcv1[:], dch1[:], op=op.subtract)
        dlo2 = io.tile([P, 1], f32)
        nc.vector.tensor_tensor(dlo2[:], dcv2[:], dch2[:], op=op.subtract)
        dcO1 = io.tile([P, P], f16)
        nc.vector.tensor_copy(dcO1[:], dch1[:].broadcast_to([P, P]))
        dcO2 = io.tile([P, P], f16)
        nc.vector.tensor_copy(dcO2[:], dch2[:].broadcast_to([P, P]))
        dcL1 = io.tile([P, P], f16)
        nc.vector.tensor_copy(dcL1[:], dlo1[:].broadcast_to([P, P]))
        dcL2 = io.tile([P, P], f16)
        nc.vector.tensor_copy(dcL2[:], dlo2[:].broadcast_to([P, P]))

        # step matrices over boundary grid
        M1 = big.tile([P, NXB], f16, tag="big")
        M2 = big.tile([P, NXB], f16, tag="big")
        for q in range(4):
            q0, q1 = NXB * q // 4, NXB * (q + 1) // 4
            nc.vector.tensor_scalar(
                M1[:, q0:q1], xb[:, q0:q1], mv1[:], None, op0=op.is_ge
            )
            nc.vector.tensor_scalar(
                M2[:, q0:q1], xb[:, q0:q1], mv2[:], None, op0=op.is_ge
            )

        # tb[j] = c0 + sum_q dc_q * M[q, j], broadcast on all partitions
        tbb = io.tile([P, NXB + 4], f32)
        # boundary j = K sits at exactly 10.0, above every midpoint
        nc.vector.tensor_copy(tbb[:, K : K + 1], cb[:, NB - 1 : NB])
        c0b = cb[:, 0:1]
        for k in range(NXB // 512):
            ps = pp.tile([P, 512], f32)
            nc.tensor.matmul(
                ps[:], dcO1[:], M1[:, 512 * k : 512 * (k + 1)],
                start=True, stop=False,
            )
            nc.tensor.matmul(
                ps[:], dcL1[:], M1[:, 512 * k : 512 * (k + 1)],
                start=False, stop=False,
            )
            nc.tensor.matmul(
                ps[:], dcO2[:], M2[:, 512 * k : 512 * (k + 1)],
                start=False, stop=False,
            )
            nc.tensor.matmul(
                ps[:], dcL2[:], M2[:, 512 * k : 512 * (k + 1)],
                start=False, stop=True,
            )
            nc.scalar.activation(
                tbb[:, 512 * k : 512 * (k + 1)], ps[:], AF.Identity,
                bias=c0b, scale=1.0,
            )

        # per-point cell index
        uf = io.tile([P, F], f32)
        nc.vector.tensor_scalar(
            uf[:], g[:], float(SCALE), -0.5, op0=op.mult, op1=op.add
        )
        u16 = io.tile([P, F], i16)
        nc.vector.tensor_scalar(
            u16[:], uf[:], float(K - 1), 0.0, op0=op.min, op1=op.max
        )

        # prep work that only needs g/mk: scheduled into the gather window
        gxp = io.tile([P, F], f32)
        nc.vector.tensor_scalar(
            gxp[:], mk[:], -BIG, BIG, op0=op.mult, op1=op.add
        )
        gx = io.tile([P, F], f32)
        nc.vector.tensor_tensor(gx[:], g[:], gxp[:], op=op.add)
        d2pen = io.tile([P, F], f32)
        nc.vector.tensor_scalar(
            d2pen[:], mk[:], -1.0e30, 1.0e30, op0=op.mult, op1=op.add
        )
        mlen = io.tile([P, 1], f32)
        nc.vector.tensor_reduce(
            mlen[:], mk[:], axis=mybir.AxisListType.X, op=op.add
        )

        # gather candidate centers tb[u], tb[u+1] in two BL-aligned f-halves;
        # each half's cham_x tail overlaps the other half's merges
        nccA = io.tile([P, 1], f32)
        nc.vector.tensor_scalar(nccA[:], ccA[:], -1.0, None, op0=op.mult)
        nccC = io.tile([P, 1], f32)
        nc.vector.tensor_scalar(nccC[:], ccC[:], -1.0, None, op0=op.mult)
        onesc = io.tile([P, 1], f32)
        nc.vector.memset(onesc[:], 1.0)

        HALVES = ((0, 216), (216, 432), (432, 600))
        NH = len(HALVES)
        ysums = io.tile([P, NH], f32)
        xmin4 = io.tile([P, 2 * NH], f32)   # column NH*b + h

        gts = []
        for f0, f1 in HALVES:
            fw = f1 - f0
            for tab0 in range(2):
                gt = big.tile([P, fw * 16], f32, tag="big")
                nc.gpsimd.ap_gather(
                    gt[:], tbb[:, tab0 : tab0 + K], u16[:, f0:f1],
                    channels=P, num_elems=K, d=1, num_idxs=fw * 16,
                )
                gts.append(gt)

        for h, (f0, f1) in enumerate(HALVES):
            fw = f1 - f0
            nb = fw // BL
            rLo = io.tile([P, fw], f32, tag=f"rlo{h}")
            nc.vector.tensor_copy(rLo[:], g[:, f0:f1])
            rHi = io.tile([P, fw], f32, tag=f"rhi{h}")
            nc.vector.tensor_copy(rHi[:], g[:, f0:f1])
            for tab0, dst in ((0, rLo), (1, rHi)):
                gv = gts[2 * h + tab0][:].rearrange("p (f r) -> p f r", r=16)
                for r in range(16):
                    nc.vector.scalar_tensor_tensor(
                        dst[:], gv[:, :, r], mneg[:, r : r + 1], dst[:],
                        op0=op.mult, op1=op.add,
                    )

            rLo2 = io.tile([P, fw], f32, tag=f"rl2{h}")
            nc.vector.tensor_tensor(rLo2[:], rLo[:], rLo[:], op=op.mult)
            rHi2 = io.tile([P, fw], f32, tag=f"rh2{h}")
            nc.vector.tensor_tensor(rHi2[:], rHi[:], rHi[:], op=op.mult)
            d2y = io.tile([P, fw], f32, tag=f"d2y{h}")
            nc.vector.tensor_tensor(d2y[:], rLo2[:], rHi2[:], op=op.min)

            junk = io.tile([P, fw], f32, tag="junk")
            nc.vector.scalar_tensor_tensor(
                junk[:], d2y[:], 1.0, mk[:, f0:f1], op0=op.mult, op1=op.mult,
                accum_out=ysums[:, h : h + 1],
            )

            d2m = io.tile([P, fw], f32, tag=f"d2m{h}")
            nc.vector.tensor_tensor(
                d2m[:], d2pen[:, f0:f1], d2y[:], op=op.add
            )
            d2mv = d2m[:].rearrange("p (b l) -> p b l", l=BL)
            gxv = gx[:, f0:f1].rearrange("p (b l) -> p b l", l=BL)
            m1t = io.tile([P, nb], f32, tag=f"m1t{h}")
            nc.vector.tensor_reduce(
                m1t[:], d2mv, axis=mybir.AxisListType.X, op=op.min
            )
            eqt = io.tile([P, fw], f32, tag=f"eqt{h}")
            eqv = eqt[:].rearrange("p (b l) -> p b l", l=BL)
            nc.vector.tensor_tensor(
                eqv, d2mv, m1t[:].unsqueeze(2).broadcast_to([P, nb, BL]),
                op=op.is_equal,
            )
            get = io.tile([P, fw], f32, tag=f"get{h}")
            gev = get[:].rearrange("p (b l) -> p b l", l=BL)
            nc.vector.tensor_tensor(gev, gxv, eqv, op=op.mult)
            gcand = io.tile([P, nb], f32, tag=f"gc{h}")
            nc.vector.tensor_reduce(
                gcand[:], gev, axis=mybir.AxisListType.X, op=op.max
            )

            off = (f0 // BL) * P
            ncand_h = nb * P
            nc.sync.dma_start(cbs_d[:, off : off + ncand_h], gcand[:])
            cbnd = io.tile([P, ncand_h], f32, tag=f"cbn{h}")
            nc.sync.dma_start(
                cbnd[:],
                cbs_d[:, off : off + ncand_h].broadcast_to([P, ncand_h]),
            )
            for b, ncc in ((0, nccA), (1, nccC)):
                d2c = big.tile([P, ncand_h], f32, tag="big")
                nc.scalar.activation(
                    d2c[:], cbnd[:], AF.Square, bias=ncc[:], scale=1.0
                )
                j = NH * b + h
                nc.vector.tensor_reduce(
                    xmin4[:, j : j + 1], d2c[:],
                    axis=mybir.AxisListType.X, op=op.min,
                )

        ysum = io.tile([P, 1], f32)
        nc.vector.tensor_reduce(
            ysum[:], ysums[:], axis=mybir.AxisListType.X, op=op.add
        )
        xmin = io.tile([P, 2], f32)
        nc.vector.tensor_reduce(
            xmin[:], xmin4[:].rearrange("p (b h) -> p b h", h=NH),
            axis=mybir.AxisListType.X, op=op.min,
        )

        # partition reductions via ones matmuls
        ps_y = pps.tile([1, 1], f32)
        nc.tensor.matmul(ps_y[:], ysum[:], onesc[:], start=True, stop=True)
        ps_m = pps.tile([1, 1], f32)
        nc.tensor.matmul(ps_m[:], mlen[:], onesc[:], start=True, stop=True)
        ps_x = pps.tile([1, 2], f32)
        nc.tensor.matmul(ps_x[:], onesc[:], xmin[:], start=True, stop=True)

        res = io.tile([1, 4], f32)
        nc.vector.memset(res[:], 0.0)
        xrow = io.tile([1, 2], f32)
        nc.vector.tensor_copy(xrow[:], ps_x[:])
        nc.vector.tensor_tensor(
            res[0:1, 0:1], xrow[0:1, 0:1], xrow[0:1, 1:2], op=op.add
        )
        nc.vector.tensor_copy(res[0:1, 1:2], ps_y[:])
        nc.vector.tensor_copy(res[0:1, 2:3], ps_m[:])
        nc.sync.dma_start(o_d[:, :], res[:])

    nc.compile()
    return nc


def _host_consts():
    xb = np.broadcast_to(
        (np.arange(NXB, dtype=np.float32) / np.float32(SCALE)).reshape(1, NXB),
        (P, NXB),
    )
    mneg = np.zeros((P, 16), dtype=np.float32)
    for p in range(P):
        mneg[p, p % 16] = -1.0
    return np.ascontiguousarray(xb), mneg


def _get_nc():
    global _NC_CACHE
    if _NC_CACHE is None:
        _NC_CACHE = _build_v2() if VERSION == 2 else _build()
    return _NC_CACHE


def kernel(depth_pred=None, depth_gt=None, depth_mask=None, bin_edges=None):
    nc = _get_nc()
    if VERSION == 2:
        xb, mneg = _host_consts()
    in_maps = []
    for n in range(NCORES):
        edges_rep = np.broadcast_to(
            bin_edges[n].reshape(1, NE).astype(np.float32), (P, NE)
        )
        im = {
            "g": np.ascontiguousarray(
                depth_gt[n].reshape(P, F).astype(np.float32)
            ),
            "mk": np.ascontiguousarray(
                depth_mask[n].reshape(P, F).astype(np.float32)
            ),
            "edges": np.ascontiguousarray(edges_rep),
        }
        if VERSION == 2:
            im["xb"] = xb
            im["mneg"] = mneg
            e = bin_edges[n].reshape(-1).astype(np.float32)
            ecol = np.empty((P, 6), dtype=np.float32)
            idx = np.arange(P)
            ecol[:, 0] = e[idx]
            ecol[:, 1] = e[idx + 1]
            ecol[:, 2] = e[idx + 2]
            ecol[:, 3] = e[np.minimum(idx + 128, NE - 2)]
            ecol[:, 4] = e[np.minimum(idx + 129, NE - 1)]
            ecol[:, 5] = e[np.minimum(idx + 130, NE - 1)]
            # pin the pad lane so ccD[127] = c_255 exactly
            ecol[127, 5] = e[255]
            im["ecol"] = ecol
        in_maps.append(im)
    res = run_bass_kernel_spmd(nc, in_maps, core_ids=list(range(NCORES)))
    per = np.empty(NCORES, dtype=np.float32)
    for n in range(NCORES):
        o = res.results[n]["out"].reshape(-1)
        per[n] = np.float32(o[0] / np.float32(NB)) + np.float32(o[1] / o[2])
    return np.float32(per.mean(dtype=np.float32))


# revision 37
# speedup vs baseline: 1.0526x; 1.0125x over previous
"""BinsChamferLoss Trainium2 Bass kernel.

Data-parallel over the batch: 8 samples -> 8 NeuronCores, one sample per core.
Each core computes its sample's chamfer terms (cham_x sum, masked cham_y sum,
valid count); the host combines the 8 per-sample scalars into the final loss.

Per-core algorithm (v1, brute force):
  points laid out [128 partitions x 600 free] (T = 76800)
  centers materialized [128 x 256] (edges host-replicated per partition)
  for each free column f: d2 = Square(centers - g[:, f]) via ACT per-partition
  bias; DVE reduce-min over centers -> cham_y column; GpSimd running min
  -> cham_x accumulator.  Invalid points are pushed to ~1e17 so they never
  win cham_x mins and their cham_y value is annihilated by the mask weight.
"""

import sys
from contextlib import ExitStack

import numpy as np

for _p in ("/opt/trn_rl_repo", "/root/.axon_site/_ro/trn_rl_repo"):
    if _p not in sys.path:
        sys.path.append(_p)

import concourse.tile as tile
from concourse import bacc, mybir
from concourse.bass_utils import run_bass_kernel_spmd

NCORES = 8
P, F = 128, 600          # per-core point layout, P*F = 76800
NB = 256                 # number of bins
NE = NB + 1              # bin edges
BIG = 1.0e17             # invalid-point displacement; BIG**2 stays finite in fp32

K = 2048                 # uniform grid cells over [0, 10)
SCALE = K / 10.0
NXB = 2048               # boundary grid built by matmul (4 x 512 PSUM chunks);
                         # tb[2048] = c_255 is patched with a copy
BL = 24                  # cham_x candidate block length (600 = 25*BL)
NBLK = F // BL
NCAND = P * NBLK         # 3200 candidates
VERSION = 2

_NC_CACHE = None


def _build():
    f32 = mybir.dt.float32
    op = mybir.AluOpType
    nc = bacc.Bacc(
        "TRN2", target_bir_lowering=False, debug=False, num_devices=NCORES
    )
    g_d = nc.dram_tensor("g", [P, F], f32, kind="ExternalInput").ap()
    m_d = nc.dram_tensor("mk", [P, F], f32, kind="ExternalInput").ap()
    e_d = nc.dram_tensor("edges", [P, NE], f32, kind="ExternalInput").ap()
    o_d = nc.dram_tensor("out", [1, 4], f32, kind="ExternalOutput").ap()

    with tile.TileContext(nc) as tc, ExitStack() as ctx:
        io = ctx.enter_context(tc.tile_pool(name="io", bufs=1))
        d2p = ctx.enter_context(tc.tile_pool(name="d2", bufs=4))

        # reload the gpsimd ucode first so it overlaps the whole table build
        nc.gpsimd.load_library(library_config.ap_gather)
        g = io.tile([P, F], f32)
        nc.sync.dma_start(g[:], g_d[:, :])
        mk = io.tile([P, F], f32)
        nc.sync.dma_start(mk[:], m_d[:, :])
        ed = io.tile([P, NE], f32)
        nc.sync.dma_start(ed[:], e_d[:, :])

        # centers = 0.5*(edges[1:] + edges[:-1]) on every partition
        cb = io.tile([P, NB], f32)
        nc.vector.tensor_tensor(cb[:], ed[:, 0:NB], ed[:, 1:NE], op=op.add)
        nc.vector.tensor_scalar_mul(cb[:], cb[:], 0.5)

        # ngx = -(mask ? g : ~BIG) = (-g) - (1-mk)*BIG, keeping the small and
        # huge scales apart so valid points stay exactly -g
        pen = io.tile([P, F], f32)
        nc.vector.tensor_scalar(
            pen[:], mk[:], -BIG, BIG, op0=op.mult, op1=op.add
        )
        ngx = io.tile([P, F], f32)
        nc.vector.scalar_tensor_tensor(
            ngx[:], g[:], -1.0, pen[:], op0=op.mult, op1=op.subtract
        )

        ymin = io.tile([P, F], f32)
        xacc = io.tile([P, NB], f32)
        nc.vector.memset(xacc[:], 3.0e38)

        for f in range(F):
            d2 = d2p.tile([P, NB], f32)
            nc.scalar.activation(
                d2[:], cb[:], mybir.ActivationFunctionType.Square,
                bias=ngx[:, f : f + 1], scale=1.0,
            )
            nc.vector.tensor_reduce(
                ymin[:, f : f + 1], d2[:], axis=mybir.AxisListType.X, op=op.min
            )
            nc.vector.tensor_tensor(xacc[:], xacc[:], d2[:], op=op.min)

        # masked cham_y sum and valid count, reduced along free dim
        wy = io.tile([P, F], f32)
        nc.vector.tensor_tensor(wy[:], ymin[:], mk[:], op=op.mult)
        ym2 = io.tile([P, 2], f32)
        nc.vector.tensor_reduce(
            ym2[:, 0:1], wy[:], axis=mybir.AxisListType.X, op=op.add
        )
        nc.vector.tensor_reduce(
            ym2[:, 1:2], mk[:], axis=mybir.AxisListType.X, op=op.add
        )

        # partition reductions on gpsimd (standard-library C-axis reduce)
        ym1 = io.tile([1, 2], f32)
        nc.gpsimd.tensor_reduce(
            ym1[:], ym2[:], axis=mybir.AxisListType.C, op=op.add
        )
        # cross-lane reduce supports only add/average/max: negate for the min
        nc.vector.tensor_scalar_mul(xacc[:], xacc[:], -1.0)
        xr = io.tile([1, NB], f32)
        nc.gpsimd.tensor_reduce(
            xr[:], xacc[:], axis=mybir.AxisListType.C, op=op.max
        )

        res = io.tile([1, 4], f32)
        nc.vector.memset(res[:], 0.0)
        nc.vector.tensor_reduce(
            res[0:1, 0:1], xr[:], axis=mybir.AxisListType.X, op=op.add,
            negate=True,
        )
        nc.vector.tensor_copy(res[0:1, 1:3], ym1[0:1, 0:2])
        nc.sync.dma_start(o_d[:, :], res[:])

    nc.compile()
    return nc


def _build_v2():
    """Grid-table kernel: nearest-center via uniform-cell two-candidate lookup.

    tb[j] = c[#midpoints <= j*delta] built as a PE matmul over the
    midpoint-vs-boundary step matrix; per-point candidates (tb[u], tb[u+1])
    fetched with one ap_gather each; cham_y = masked sum of min residual^2.
    cham_x: per-(partition, block) argmin candidates of the masked residuals,
    then exact 256 x NCAND brute force.
    """
    f32 = mybir.dt.float32
    i16 = mybir.dt.int16
    op = mybir.AluOpType
    AF = mybir.ActivationFunctionType
    from concourse import library_config

    nc = bacc.Bacc(
        "TRN2", target_bir_lowering=False, debug=False, num_devices=NCORES
    )
    g_d = nc.dram_tensor("g", [P, F], f32, kind="ExternalInput").ap()
    m_d = nc.dram_tensor("mk", [P, F], f32, kind="ExternalInput").ap()
    e_d = nc.dram_tensor("edges", [P, NE], f32, kind="ExternalInput").ap()
    xb_d = nc.dram_tensor("xb", [P, NXB], f32, kind="ExternalInput").ap()
    mn_d = nc.dram_tensor("mneg", [P, 16], f32, kind="ExternalInput").ap()
    ec_d = nc.dram_tensor("ecol", [P, 6], f32, kind="ExternalInput").ap()
    o_d = nc.dram_tensor("out", [1, 4], f32, kind="ExternalOutput").ap()
    cbs_d = nc.dram_tensor("cbs", [1, NCAND], f32).ap()

    with tile.TileContext(nc) as tc, ExitStack() as ctx:
        io = ctx.enter_context(tc.tile_pool(name="io", bufs=1))
        big = ctx.enter_context(tc.tile_pool(name="big", bufs=3))
        pp = ctx.enter_context(tc.tile_pool(name="pp", bufs=4, space="PSUM"))
        pps = ctx.enter_context(tc.tile_pool(name="pps", bufs=1, space="PSUM"))

        # reload the gpsimd ucode first so it overlaps the whole table build
        nc.gpsimd.load_library(library_config.ap_gather)
        # table-build inputs first: the SP sequencer issues DMAs serially
        # (~565ns each) and ecol/xb gate the critical chain
        ec = io.tile([P, 6], f32)
        nc.sync.dma_start(ec[:], ec_d[:, :])
        xb = big.tile([P, NXB], f32, tag="big")
        for q in range(4):
            q0, q1 = NXB * q // 4, NXB * (q + 1) // 4
            nc.sync.dma_start(xb[:, q0:q1], xb_d[:, q0:q1])
        ed = io.tile([P, NE], f32)
        nc.sync.dma_start(ed[:], e_d[:, :])
        g = io.tile([P, F], f32)
        nc.sync.dma_start(g[:], g_d[:, :])
        mk = io.tile([P, F], f32)
        nc.sync.dma_start(mk[:], m_d[:, :])
        mneg = io.tile([P, 16], f32)
        nc.sync.dma_start(mneg[:], mn_d[:, :])

        # centers on every partition
        cb = io.tile([P, NB], f32)
        nc.vector.tensor_tensor(cb[:], ed[:, 0:NB], ed[:, 1:NE], op=op.add)
        nc.vector.tensor_scalar_mul(cb[:], cb[:], 0.5)

        # per-partition center columns from the host-transposed edge columns
        ccA = io.tile([P, 1], f32)   # c_0..127
        nc.vector.tensor_tensor(ccA[:], ec[:, 0:1], ec[:, 1:2], op=op.add)
        nc.vector.tensor_scalar_mul(ccA[:], ccA[:], 0.5)
        ccB = io.tile([P, 1], f32)   # c_1..128
        nc.vector.tensor_tensor(ccB[:], ec[:, 1:2], ec[:, 2:3], op=op.add)
        nc.vector.tensor_scalar_mul(ccB[:], ccB[:], 0.5)
        ccC = io.tile([P, 1], f32)   # c_128..255
        nc.vector.tensor_tensor(ccC[:], ec[:, 3:4], ec[:, 4:5], op=op.add)
        nc.vector.tensor_scalar_mul(ccC[:], ccC[:], 0.5)
        ccD = io.tile([P, 1], f32)   # c_129..255, last lane pinned to c_255
        nc.vector.tensor_tensor(ccD[:], ec[:, 4:5], ec[:, 5:6], op=op.add)
        nc.vector.tensor_scalar_mul(ccD[:], ccD[:], 0.5)

        # midpoints and center deltas per partition (two 128-blocks)
        mv1 = io.tile([P, 1], f32)
        nc.vector.tensor_tensor(mv1[:], ccA[:], ccB[:], op=op.add)
        nc.vector.tensor_scalar_mul(mv1[:], mv1[:], 0.5)
        mv2 = io.tile([P, 1], f32)
        nc.vector.tensor_tensor(mv2[:], ccC[:], ccD[:], op=op.add)
        nc.vector.tensor_scalar_mul(mv2[:], mv2[:], 0.5)
        dcv1 = io.tile([P, 1], f32)
        nc.vector.tensor_tensor(dcv1[:], ccB[:], ccA[:], op=op.subtract)
        # dcv2[127] = c_255 - c_255 = 0, so the padded midpoint row is inert
        dcv2 = io.tile([P, 1], f32)
        nc.vector.tensor_tensor(dcv2[:], ccD[:], ccC[:], op=op.subtract)

        # fp16 matmul with Dekker hi/lo split of dc so the 255-term prefix
        # sums stay fp32-accurate while the matmul runs at fp16 rate
        f16 = mybir.dt.float16
        dch1 = io.tile([P, 1], f16)
        nc.vector.tensor_copy(dch1[:], dcv1[:])
        dch2 = io.tile([P, 1], f16)
        nc.vector.tensor_copy(dch2[:], dcv2[:])
        dlo1 = io.tile([P, 1], f32)
        nc.vector.tensor_tensor(dlo1[:], dcv1[:], dch1[:], op=op.subtract)
        dlo2 = io.tile([P, 1], f32)
        nc.vector.tensor_tensor(dlo2[:], dcv2[:], dch2[:], op=op.subtract)
        dcO1 = io.tile([P, P], f16)
        nc.vector.tensor_copy(dcO1[:], dch1[:].broadcast_to([P, P]))
        dcO2 = io.tile([P, P], f16)
        nc.vector.tensor_copy(dcO2[:], dch2[:].broadcast_to([P, P]))
        dcL1 = io.tile([P, P], f16)
        nc.vector.tensor_copy(dcL1[:], dlo1[:].broadcast_to([P, P]))
        dcL2 = io.tile([P, P], f16)
        nc.vector.tensor_copy(dcL2[:], dlo2[:].broadcast_to([P, P]))

        # step matrices over boundary grid
        M1 = big.tile([P, NXB], f16, tag="big")
        M2 = big.tile([P, NXB], f16, tag="big")
        for q in range(4):
            q0, q1 = NXB * q // 4, NXB * (q + 1) // 4
            nc.gpsimd.tensor_scalar(
                M1[:, q0:q1], xb[:, q0:q1], mv1[:], None, op0=op.is_ge
            )
            nc.gpsimd.tensor_scalar(
                M2[:, q0:q1], xb[:, q0:q1], mv2[:], None, op0=op.is_ge
            )

        # tb[j] = c0 + sum_q dc_q * M[q, j], broadcast on all partitions
        tbb = io.tile([P, NXB + 4], f32)
        # boundary j = K sits at exactly 10.0, above every midpoint
        nc.vector.tensor_copy(tbb[:, K : K + 1], cb[:, NB - 1 : NB])
        c0b = cb[:, 0:1]
        for k in range(NXB // 512):
            ps = pp.tile([P, 512], f32)
            nc.tensor.matmul(
                ps[:], dcO1[:], M1[:, 512 * k : 512 * (k + 1)],
                start=True, stop=False,
            )
            nc.tensor.matmul(
                ps[:], dcL1[:], M1[:, 512 * k : 512 * (k + 1)],
                start=False, stop=False,
            )
            nc.tensor.matmul(
                ps[:], dcO2[:], M2[:, 512 * k : 512 * (k + 1)],
                start=False, stop=False,
            )
            nc.tensor.matmul(
                ps[:], dcL2[:], M2[:, 512 * k : 512 * (k + 1)],
                start=False, stop=True,
            )
            nc.scalar.activation(
                tbb[:, 512 * k : 512 * (k + 1)], ps[:], AF.Identity,
                bias=c0b, scale=1.0,
            )

        # per-point cell index
        uf = io.tile([P, F], f32)
        nc.vector.tensor_scalar(
            uf[:], g[:], float(SCALE), -0.5, op0=op.mult, op1=op.add
        )
        u16 = io.tile([P, F], i16)
        nc.vector.tensor_scalar(
            u16[:], uf[:], float(K - 1), 0.0, op0=op.min, op1=op.max
        )

        # prep work that only needs g/mk: scheduled into the gather window
        gxp = io.tile([P, F], f32)
        nc.vector.tensor_scalar(
            gxp[:], mk[:], -BIG, BIG, op0=op.mult, op1=op.add
        )
        gx = io.tile([P, F], f32)
        nc.vector.tensor_tensor(gx[:], g[:], gxp[:], op=op.add)
        d2pen = io.tile([P, F], f32)
        nc.vector.tensor_scalar(
            d2pen[:], mk[:], -1.0e30, 1.0e30, op0=op.mult, op1=op.add
        )
        mlen = io.tile([P, 1], f32)
        nc.vector.tensor_reduce(
            mlen[:], mk[:], axis=mybir.AxisListType.X, op=op.add
        )

        # gather candidate centers tb[u], tb[u+1] in two BL-aligned f-halves;
        # each half's cham_x tail overlaps the other half's merges
        nccA = io.tile([P, 1], f32)
        nc.vector.tensor_scalar(nccA[:], ccA[:], -1.0, None, op0=op.mult)
        nccC = io.tile([P, 1], f32)
        nc.vector.tensor_scalar(nccC[:], ccC[:], -1.0, None, op0=op.mult)
        onesc = io.tile([P, 1], f32)
        nc.vector.memset(onesc[:], 1.0)

        HALVES = ((0, 216), (216, 432), (432, 600))
        NH = len(HALVES)
        ysums = io.tile([P, NH], f32)
        xmin4 = io.tile([P, 2 * NH], f32)   # column NH*b + h

        gts = []
        for f0, f1 in HALVES:
            fw = f1 - f0
            for tab0 in range(2):
                gt = big.tile([P, fw * 16], f32, tag="big")
                nc.gpsimd.ap_gather(
                    gt[:], tbb[:, tab0 : tab0 + K], u16[:, f0:f1],
                    channels=P, num_elems=K, d=1, num_idxs=fw * 16,
                )
                gts.append(gt)

        for h, (f0, f1) in enumerate(HALVES):
            fw = f1 - f0
            nb = fw // BL
            rLo = io.tile([P, fw], f32, tag=f"rlo{h}")
            nc.vector.tensor_copy(rLo[:], g[:, f0:f1])
            rHi = io.tile([P, fw], f32, tag=f"rhi{h}")
            nc.vector.tensor_copy(rHi[:], g[:, f0:f1])
            for tab0, dst in ((0, rLo), (1, rHi)):
                gv = gts[2 * h + tab0][:].rearrange("p (f r) -> p f r", r=16)
                for r in range(16):
                    nc.vector.scalar_tensor_tensor(
                        dst[:], gv[:, :, r], mneg[:, r : r + 1], dst[:],
                        op0=op.mult, op1=op.add,
                    )

            rLo2 = io.tile([P, fw], f32, tag=f"rl2{h}")
            nc.vector.tensor_tensor(rLo2[:], rLo[:], rLo[:], op=op.mult)
            rHi2 = io.tile([P, fw], f32, tag=f"rh2{h}")
            nc.vector.tensor_tensor(rHi2[:], rHi[:], rHi[:], op=op.mult)
            d2y = io.tile([P, fw], f32, tag=f"d2y{h}")
            nc.vector.tensor_tensor(d2y[:], rLo2[:], rHi2[:], op=op.min)

            junk = io.tile([P, fw], f32, tag="junk")
            nc.vector.scalar_tensor_tensor(
                junk[:], d2y[:], 1.0, mk[:, f0:f1], op0=op.mult, op1=op.mult,
                accum_out=ysums[:, h : h + 1],
            )

            d2m = io.tile([P, fw], f32, tag=f"d2m{h}")
            nc.vector.tensor_tensor(
                d2m[:], d2pen[:, f0:f1], d2y[:], op=op.add
            )
            d2mv = d2m[:].rearrange("p (b l) -> p b l", l=BL)
            gxv = gx[:, f0:f1].rearrange("p (b l) -> p b l", l=BL)
            m1t = io.tile([P, nb], f32, tag=f"m1t{h}")
            nc.vector.tensor_reduce(
                m1t[:], d2mv, axis=mybir.AxisListType.X, op=op.min
            )
            eqt = io.tile([P, fw], f32, tag=f"eqt{h}")
            eqv = eqt[:].rearrange("p (b l) -> p b l", l=BL)
            nc.vector.tensor_tensor(
                eqv, d2mv, m1t[:].unsqueeze(2).broadcast_to([P, nb, BL]),
                op=op.is_equal,
            )
            get = io.tile([P, fw], f32, tag=f"get{h}")
            gev = get[:].rearrange("p (b l) -> p b l", l=BL)
            nc.vector.tensor_tensor(gev, gxv, eqv, op=op.mult)
            gcand = io.tile([P, nb], f32, tag=f"gc{h}")
            nc.vector.tensor_reduce(
                gcand[:], gev, axis=mybir.AxisListType.X, op=op.max
            )

            off = (f0 // BL) * P
            ncand_h = nb * P
            nc.sync.dma_start(cbs_d[:, off : off + ncand_h], gcand[:])
            cbnd = io.tile([P, ncand_h], f32, tag=f"cbn{h}")
            nc.sync.dma_start(
                cbnd[:],
                cbs_d[:, off : off + ncand_h].broadcast_to([P, ncand_h]),
            )
            for b, ncc in ((0, nccA), (1, nccC)):
                d2c = big.tile([P, ncand_h], f32, tag="big")
                nc.scalar.activation(
                    d2c[:], cbnd[:], AF.Square, bias=ncc[:], scale=1.0
                )
                j = NH * b + h
                nc.vector.tensor_reduce(
                    xmin4[:, j : j + 1], d2c[:],
                    axis=mybir.AxisListType.X, op=op.min,
                )

        ysum = io.tile([P, 1], f32)
        nc.vector.tensor_reduce(
            ysum[:], ysums[:], axis=mybir.AxisListType.X, op=op.add
        )
        xmin = io.tile([P, 2], f32)
        nc.vector.tensor_reduce(
            xmin[:], xmin4[:].rearrange("p (b h) -> p b h", h=NH),
            axis=mybir.AxisListType.X, op=op.min,
        )

        # partition reductions via ones matmuls
        ps_y = pps.tile([1, 1], f32)
        nc.tensor.matmul(ps_y[:], ysum[:], onesc[:], start=True, stop=True)
        ps_m = pps.tile([1, 1], f32)
        nc.tensor.matmul(ps_m[:], mlen[:], onesc[:], start=True, stop=True)
        ps_x = pps.tile([1, 2], f32)
        nc.tensor.matmul(ps_x[:], onesc[:], xmin[:], start=True, stop=True)

        res = io.tile([1, 4], f32)
        nc.vector.memset(res[:], 0.0)
        xrow = io.tile([1, 2], f32)
        nc.vector.tensor_copy(xrow[:], ps_x[:])
        nc.vector.tensor_tensor(
            res[0:1, 0:1], xrow[0:1, 0:1], xrow[0:1, 1:2], op=op.add
        )
        nc.vector.tensor_copy(res[0:1, 1:2], ps_y[:])
        nc.vector.tensor_copy(res[0:1, 2:3], ps_m[:])
        nc.sync.dma_start(o_d[:, :], res[:])

    nc.compile()
    return nc


def _host_consts():
    xb = np.broadcast_to(
        (np.arange(NXB, dtype=np.float32) / np.float32(SCALE)).reshape(1, NXB),
        (P, NXB),
    )
    mneg = np.zeros((P, 16), dtype=np.float32)
    for p in range(P):
        mneg[p, p % 16] = -1.0
    return np.ascontiguousarray(xb), mneg


def _get_nc():
    global _NC_CACHE
    if _NC_CACHE is None:
        _NC_CACHE = _build_v2() if VERSION == 2 else _build()
    return _NC_CACHE


def kernel(depth_pred=None, depth_gt=None, depth_mask=None, bin_edges=None):
    nc = _get_nc()
    if VERSION == 2:
        xb, mneg = _host_consts()
    in_maps = []
    for n in range(NCORES):
        edges_rep = np.broadcast_to(
            bin_edges[n].reshape(1, NE).astype(np.float32), (P, NE)
        )
        im = {
            "g": np.ascontiguousarray(
                depth_gt[n].reshape(P, F).astype(np.float32)
            ),
            "mk": np.ascontiguousarray(
                depth_mask[n].reshape(P, F).astype(np.float32)
            ),
            "edges": np.ascontiguousarray(edges_rep),
        }
        if VERSION == 2:
            im["xb"] = xb
            im["mneg"] = mneg
            e = bin_edges[n].reshape(-1).astype(np.float32)
            ecol = np.empty((P, 6), dtype=np.float32)
            idx = np.arange(P)
            ecol[:, 0] = e[idx]
            ecol[:, 1] = e[idx + 1]
            ecol[:, 2] = e[idx + 2]
            ecol[:, 3] = e[np.minimum(idx + 128, NE - 2)]
            ecol[:, 4] = e[np.minimum(idx + 129, NE - 1)]
            ecol[:, 5] = e[np.minimum(idx + 130, NE - 1)]
            # pin the pad lane so ccD[127] = c_255 exactly
            ecol[127, 5] = e[255]
            im["ecol"] = ecol
        in_maps.append(im)
    res = run_bass_kernel_spmd(nc, in_maps, core_ids=list(range(NCORES)))
    per = np.empty(NCORES, dtype=np.float32)
    for n in range(NCORES):
        o = res.results[n]["out"].reshape(-1)
        per[n] = np.float32(o[0] / np.float32(NB)) + np.float32(o[1] / o[2])
    return np.float32(per.mean(dtype=np.float32))
